# revision 12
# baseline (speedup 1.0000x reference)
"""PhysicsInformedLoss on 8 Trainium2 NeuronCores.

Sharding strategy (degree-class padded CSR):
- Edges are grouped by destination node `row` (the scatter target of every
  segment-mean in the reference). Nodes with deg>0 are binned into degree
  classes K (multiples of 4, small classes merged); each node gets exactly K
  contiguous "slots" (its edges + self-pads, pads contribute exactly 0).
- Nodes of each class are split evenly across the 8 cores (identical padded
  per-core counts -> one SPMD program). Per core, node i of a class maps to
  partition p = i // q (q nodes/partition), so every per-node segment sum is
  a static strided reduction along the free dimension.
- The host gathers the 7 col-side planes (pos xyz, vel uvw, p) in slot order
  (this is the "shard the edges" data layout step); the row side is the
  per-node resident plane broadcast along K by a stride-0 access pattern.
- Device per core: stream col planes, compute all per-edge terms, strided
  per-node reduce, finish div/residual/squares; output per-partition partial
  sums. Host sums 8x128 partials and forms the scalar loss.
"""
import contextlib
import ctypes
import os
import sys
import tempfile
import types

import numpy as np

import concourse.bass as bass
import concourse.tile as tile
from concourse import mybir
from concourse.vector_clock import ScopedClock
from concourse.bass_utils import run_bass_kernel_spmd

N_CORES = 8
P = 128
EPS = 1e-8
REYNOLDS = 1000000.0
LAMBDA_CONT = 0.1
LAMBDA_MOM = 0.01
F_TILE = 768  # target per-partition columns per tile

# ---------------------------------------------------------------- tile patch
# walrus in this environment allows only ONE sync-wait per instruction, but
# Tile's scheduler can emit several. Split surplus waits onto engine NOPs
# inserted right before the offending instruction.
_MAX_WAITS = 1


def _split_multi_waits(nc, handles):
    work = []
    for fn in nc.m.functions:
        for bb in fn.blocks:
            items = []
            for inst in bb.instructions:
                si = inst.sync_info
                waits = list(si.on_wait) if si and si.on_wait else []
                if len(waits) > _MAX_WAITS:
                    keep = len(waits) - _MAX_WAITS
                    extra = waits[:keep]
                    si.on_wait = waits[keep:]
                    chunks = [
                        extra[i : i + _MAX_WAITS]
                        for i in range(0, len(extra), _MAX_WAITS)
                    ]
                    items.append((inst.name, inst.engine, chunks))
            if items:
                work.append((bb, items))
    if not work:
        return
    created = {}
    placements = {}
    for bb, items in work:
        plc = {}
        for inst_name, engine, chunks in items:
            nops = []
            for chunk in chunks:
                ni = nc.engines[engine].nop(nofuse=True)
                for w in chunk:
                    h = handles.get(w.ant_name)
                    assert h is not None, f"no sem handle for {w.ant_name}"
                    ni.wait_op(h, w.wait_value, "sem-ge")
                created[ni.ins.name] = None
                nops.append(ni.ins)
            plc[inst_name] = nops
        placements[id(bb)] = plc
    for fn in nc.m.functions:
        for bb in fn.blocks:
            plc = placements.get(id(bb), {})
            newlist = []
            for inst in bb.instructions:
                if inst.name in created:
                    continue
                if inst.name in plc:
                    newlist.extend(plc[inst.name])
                newlist.append(inst)
            bb.instructions = newlist


def _patched_drain_and_barrier(self, tick_clock, wait_clock):
    drain_inst = self.nc.sync.drain()
    wait_clock.add_sem_waits(
        drain_inst.ins, ScopedClock({None: tick_clock.global_clock})
    )
    handles = {h.name: h for h in self.sems.allocated().values()}
    _split_multi_waits(self.nc, handles)
    self.nc.all_engine_barrier()
    popped = self.nc._tile_sem_poison_stack.pop()
    assert popped is self._sem_poison
    self.nc.clear_and_free_semaphores(list(self.sems.allocated().values()))
    self.nc.all_engine_barrier()


tile.TileContext._drain_and_barrier = _patched_drain_and_barrier

# ------------------------------------------------------------- ntff hook
# The env's antenv package lacks axon_hooks; recreate the NTFF profile hook
# via ctypes so run_bass_kernel_spmd(trace=True) works (test/profiling only).
_AXON_SO = "/opt/axon/libaxon_pjrt.so"


def _install_ntff_hook():
    if "antenv.axon_hooks" in sys.modules:
        return
    try:
        lib = ctypes.CDLL(_AXON_SO)
        lib.axon_start_nrt_profile.argtypes = [
            ctypes.POINTER(ctypes.c_int64),
            ctypes.c_size_t,
        ]
        lib.axon_start_nrt_profile.restype = ctypes.c_int64
        lib.axon_stop_nrt_profile.argtypes = [ctypes.c_char_p]
        lib.axon_stop_nrt_profile.restype = ctypes.c_int64
    except Exception:
        return

    @contextlib.contextmanager
    def _hook(output_dir, device_ids):
        import jax

        jax.devices()
        if device_ids:
            ids = (ctypes.c_int64 * len(device_ids))(*device_ids)
            rc = lib.axon_start_nrt_profile(ids, len(device_ids))
        else:
            rc = lib.axon_start_nrt_profile(None, 0)
        if rc != 0:
            raise RuntimeError(f"axon_start_nrt_profile rc={rc}")
        try:
            yield
        finally:
            n = lib.axon_stop_nrt_profile(str(output_dir).encode())
            print(f"profile: {n} file(s) written to {output_dir}", file=sys.stderr)

    mod = types.ModuleType("antenv.axon_hooks")
    mod.get_axon_ntff_profile_hook = lambda: _hook
    mod.set_axon_ntff_profile_hook = lambda h: None
    sys.modules["antenv.axon_hooks"] = mod


# ---------------------------------------------------------------- host prep


def _build_plan(row, n, min_class_slots=128 * 1024):
    deg = np.bincount(row, minlength=n).astype(np.int64)
    order = np.argsort(row, kind="stable")
    offs = np.zeros(n + 1, dtype=np.int64)
    np.cumsum(deg, out=offs[1:])

    kraw = ((np.maximum(deg, 1) + 3) // 4) * 4
    active = deg > 0
    uniq = np.unique(kraw[active])
    classes = []
    pend = []
    pend_slots = 0
    for K in uniq:
        ids = np.nonzero(active & (kraw == K))[0]
        pend.append(ids)
        pend_slots += ids.size * int(K)
        if pend_slots >= min_class_slots or K == uniq[-1]:
            allids = np.concatenate(pend)
            classes.append((int(K), allids))
            pend = []
            pend_slots = 0
    plan = []
    for K, ids in classes:
        m = -(-ids.size // (N_CORES * P)) * P
        plan.append((K, ids, m))
    return plan, deg, offs, order


def _build_streams(plan, deg, offs, col_sorted, nodedata):
    """nodedata: [n,7] f32. Returns (cores list of dicts, NN, S)."""
    S = sum(m * K for K, _, m in plan)
    NN = sum(m for _, _, m in plan)
    cores = []
    for c in range(N_CORES):
        col_planes = np.zeros((7, P, S // P), np.float32)
        node_planes = np.zeros((7, P, NN // P), np.float32)
        cnt = np.ones((P, NN // P), np.float32)
        off_slots = 0
        off_nodes = 0
        for K, ids, m in plan:
            q = m // P
            ids_c = ids[c * m : (c + 1) * m]
            k_real = ids_c.size
            vals = np.zeros((m, K, 7), np.float32)
            if k_real > 0:
                colmat = np.empty((k_real, K), np.int64)
                colmat[:] = ids_c[:, None]
                dd = deg[ids_c]
                oo = offs[ids_c]
                ar = np.arange(K)[None, :]
                valid = ar < dd[:, None]
                src_idx = (oo[:, None] + ar)[valid]
                colmat[valid] = col_sorted[src_idx]
                vals[:k_real] = nodedata[colmat]
                nodevals = np.zeros((m, 7), np.float32)
                nodevals[:k_real] = nodedata[ids_c]
                cv = np.ones(m, np.float32)
                cv[:k_real] = np.maximum(dd, 1).astype(np.float32)
            else:
                nodevals = np.zeros((m, 7), np.float32)
                cv = np.ones(m, np.float32)
            col_planes[:, :, off_slots : off_slots + q * K] = vals.reshape(
                P, q, K, 7
            ).transpose(3, 0, 1, 2).reshape(7, P, q * K)
            node_planes[:, :, off_nodes : off_nodes + q] = nodevals.reshape(
                P, q, 7
            ).transpose(2, 0, 1)
            cnt[:, off_nodes : off_nodes + q] = cv.reshape(P, q)
            off_slots += q * K
            off_nodes += q
        cores.append(dict(col=col_planes, nod=node_planes, cnt=cnt))
    return cores, NN, S


# ---------------------------------------------------------------- bass build


def _class_tiles(plan):
    """Yield (K, q_nodes_in_tile, slot_col_offset, node_col_offset) splits."""
    tiles = []
    off_s = 0
    off_n = 0
    for K, _, m in plan:
        q = m // P
        # split q nodes into groups of ~F_TILE/K
        gmax = max(1, F_TILE // K)
        i = 0
        while i < q:
            g = min(gmax, q - i)
            tiles.append((K, g, off_s + i * K, off_n + i))
            i += g
        off_s += q * K
        off_n += q
    return tiles


def _build_nc(plan, NN, S, DQ):
    """Build the SPMD bass program. DQ = per-partition cols of data-loss
    planes (4 planes each for pred/target slices)."""
    fp32 = mybir.dt.float32
    nc = bass.Bass("TRN2", target_bir_lowering=False)
    W = S // P
    Q = NN // P

    cols = [
        nc.dram_tensor(f"col{i}", [P, W], fp32, kind="ExternalInput")
        for i in range(7)
    ]
    nod = nc.dram_tensor("nod", [P, 7 * Q], fp32, kind="ExternalInput")
    cntT = nc.dram_tensor("cnt", [P, Q], fp32, kind="ExternalInput")
    dlp = nc.dram_tensor("dlp", [P, 4 * DQ], fp32, kind="ExternalInput")
    dlt = nc.dram_tensor("dlt", [P, 4 * DQ], fp32, kind="ExternalInput")
    out = nc.dram_tensor("out", [P, 8], fp32, kind="ExternalOutput")

    AF = mybir.ActivationFunctionType
    OP = mybir.AluOpType

    with tile.TileContext(nc) as tc:
        with (
            tc.tile_pool(name="resident", bufs=1) as res_pool,
            tc.tile_pool(name="colp", bufs=3) as col_pool,
            tc.tile_pool(name="tmp", bufs=2) as tmp_pool,
        ):
            # resident: node planes, cnt, accumulators
            nodt = res_pool.tile([P, 7 * Q], fp32)
            nc.sync.dma_start(nodt[:], nod.ap()[:])
            cntt = res_pool.tile([P, Q], fp32)
            nc.sync.dma_start(cntt[:], cntT.ap()[:])
            acc = res_pool.tile([P, 7 * Q], fp32)  # g, lx, ly, lz, px, py, pz

            # ---- data loss (small) ----
            dlpt = res_pool.tile([P, 4 * DQ], fp32)
            nc.sync.dma_start(dlpt[:], dlp.ap()[:])
            dltt = res_pool.tile([P, 4 * DQ], fp32)
            nc.sync.dma_start(dltt[:], dlt.ap()[:])
            dld = res_pool.tile([P, 4 * DQ], fp32)
            nc.vector.tensor_sub(dld[:], dlpt[:], dltt[:])
            acc_vel = res_pool.tile([P, 1], fp32)
            acc_pres = res_pool.tile([P, 1], fp32)
            nc.scalar.activation(
                dld[:, 0 : 3 * DQ], dld[:, 0 : 3 * DQ], AF.Square,
                accum_out=acc_vel[:],
            )
            nc.scalar.activation(
                dld[:, 3 * DQ : 4 * DQ], dld[:, 3 * DQ : 4 * DQ], AF.Square,
                accum_out=acc_pres[:],
            )

            # ---- main loop ----
            for K, g, off_s, off_n in _class_tiles(plan):
                F = g * K
                ct = [
                    col_pool.tile([P, F], fp32, tag=f"c{i}", name=f"ct{i}")
                    for i in range(7)
                ]
                for i in range(7):
                    nc.sync.dma_start(ct[i][:], cols[i].ap()[:, off_s : off_s + F])

                def nview(i):
                    # node plane i slice broadcast along K: [P, g, K]
                    a = nodt[:, i * Q + off_n : i * Q + off_n + g]
                    return a.unsqueeze(-1).broadcast_to([P, g, K])

                def v3(t):
                    return t[:].rearrange("p (g k) -> p g k", g=g, k=K)

                dx = tmp_pool.tile([P, F], fp32, tag="dx")
                dy = tmp_pool.tile([P, F], fp32, tag="dy")
                dz = tmp_pool.tile([P, F], fp32, tag="dz")
                nc.vector.tensor_sub(v3(dx), v3(ct[0]), nview(0))
                nc.vector.tensor_sub(v3(dy), v3(ct[1]), nview(1))
                nc.vector.tensor_sub(v3(dz), v3(ct[2]), nview(2))

                r2 = tmp_pool.tile([P, F], fp32, tag="r2")
                tq = tmp_pool.tile([P, F], fp32, tag="tq")
                nc.vector.tensor_mul(r2[:], dx[:], dx[:])
                nc.vector.tensor_mul(tq[:], dy[:], dy[:])
                nc.vector.tensor_add(r2[:], r2[:], tq[:])
                nc.vector.tensor_mul(tq[:], dz[:], dz[:])
                nc.vector.tensor_add(r2[:], r2[:], tq[:])

                # d1 = 1/(sqrt(r2)+eps)
                d1 = tmp_pool.tile([P, F], fp32, tag="d1")
                nc.scalar.sqrt(d1[:], r2[:])
                nc.vector.tensor_scalar_add(d1[:], d1[:], EPS)
                nc.vector.reciprocal(d1[:], d1[:])
                # d2 = 1/(r2+eps)
                d2 = tmp_pool.tile([P, F], fp32, tag="d2")
                nc.vector.tensor_scalar_add(r2[:], r2[:], EPS)
                nc.vector.reciprocal(d2[:], r2[:])

                du = tmp_pool.tile([P, F], fp32, tag="du")
                dv = tmp_pool.tile([P, F], fp32, tag="dv")
                dw = tmp_pool.tile([P, F], fp32, tag="dw")
                nc.vector.tensor_sub(v3(du), v3(ct[3]), nview(3))
                nc.vector.tensor_sub(v3(dv), v3(ct[4]), nview(4))
                nc.vector.tensor_sub(v3(dw), v3(ct[5]), nview(5))

                gg = tmp_pool.tile([P, F], fp32, tag="gg")
                nc.vector.tensor_mul(gg[:], du[:], dx[:])
                nc.vector.tensor_mul(tq[:], dv[:], dy[:])
                nc.vector.tensor_add(gg[:], gg[:], tq[:])
                nc.vector.tensor_mul(tq[:], dw[:], dz[:])
                nc.vector.tensor_add(gg[:], gg[:], tq[:])
                nc.vector.tensor_mul(gg[:], gg[:], d1[:])  # velgrad

                dqq = tmp_pool.tile([P, F], fp32, tag="dqq")
                nc.vector.tensor_sub(v3(dqq), v3(ct[6]), nview(6))
                nc.vector.tensor_mul(dqq[:], dqq[:], d1[:])
                nc.vector.tensor_mul(dqq[:], dqq[:], d1[:])  # cp = dq/dist^2

                # pg into dx,dy,dz ; lap into du,dv,dw
                nc.vector.tensor_mul(dx[:], dqq[:], dx[:])
                nc.vector.tensor_mul(dy[:], dqq[:], dy[:])
                nc.vector.tensor_mul(dz[:], dqq[:], dz[:])
                nc.vector.tensor_mul(du[:], du[:], d2[:])
                nc.vector.tensor_mul(dv[:], dv[:], d2[:])
                nc.vector.tensor_mul(dw[:], dw[:], d2[:])

                # per-node segment sums -> acc slices
                for idx, t in enumerate([gg, du, dv, dw, dx, dy, dz]):
                    nc.vector.tensor_reduce(
                        acc[:, idx * Q + off_n : idx * Q + off_n + g],
                        v3(t),
                        mybir.AxisListType.X,
                        OP.add,
                    )

            # ---- finish ----
            icnt = res_pool.tile([P, Q], fp32)
            nc.vector.reciprocal(icnt[:], cntt[:])
            div = res_pool.tile([P, Q], fp32)
            nc.vector.tensor_mul(div[:], acc[:, 0:Q], icnt[:])
            acc_div2 = res_pool.tile([P, 1], fp32)
            nc.scalar.activation(div[:], div[:], AF.Square, accum_out=acc_div2[:])
            acc_m = [
                res_pool.tile([P, 1], fp32, tag=f"am{i}", name=f"am{i}")
                for i in range(3)
            ]
            for i in range(3):
                r = res_pool.tile([P, Q], fp32, tag="rfin")
                nc.vector.scalar_tensor_tensor(
                    r[:],
                    acc[:, (1 + i) * Q : (2 + i) * Q],
                    1.0 / REYNOLDS,
                    acc[:, (4 + i) * Q : (5 + i) * Q],
                    OP.mult,
                    OP.add,
                )
                nc.vector.tensor_mul(r[:], r[:], icnt[:])
                nc.scalar.activation(r[:], r[:], AF.Square, accum_out=acc_m[i][:])

            outt = res_pool.tile([P, 8], fp32)
            nc.vector.memset(outt[:], 0.0)
            nc.vector.tensor_copy(outt[:, 0:1], acc_vel[:])
            nc.vector.tensor_copy(outt[:, 1:2], acc_pres[:])
            nc.vector.tensor_copy(outt[:, 2:3], acc_div2[:])
            nc.vector.tensor_copy(outt[:, 3:4], acc_m[0][:])
            nc.vector.tensor_copy(outt[:, 4:5], acc_m[1][:])
            nc.vector.tensor_copy(outt[:, 5:6], acc_m[2][:])
            nc.sync.dma_start(out.ap()[:], outt[:])

    return nc


# ---------------------------------------------------------------- entry

_CACHE = {}


def _get_nc(key, plan, NN, S, DQ):
    if key not in _CACHE:
        _CACHE[key] = _build_nc(plan, NN, S, DQ)
    return _CACHE[key]


LAST_RESULT = None  # BassKernelResults of the most recent run (for profiling)


def kernel(pred, target, edge_index, pos, _trace_dir=None):
    global LAST_RESULT
    pred = np.asarray(pred)
    target = np.asarray(target)
    pos = np.asarray(pos)
    row = np.asarray(edge_index[0]).astype(np.int64)
    col = np.asarray(edge_index[1]).astype(np.int64)
    n = pred.shape[0]

    plan, deg, offs, order = _build_plan(row, n)
    col_sorted = col[order]
    nodedata = np.concatenate(
        [pos.astype(np.float32), pred.astype(np.float32)], axis=1
    )
    cores, NN, S = _build_streams(plan, deg, offs, col_sorted, nodedata)

    # data-loss slices: split all n nodes across cores, pad to mult of 128
    per = -(-n // N_CORES)
    DQ = (-(-per // P) * P) // P
    predf = pred.astype(np.float32)
    targf = target.astype(np.float32)

    in_maps = []
    for c in range(N_CORES):
        lo, hi = c * per, min((c + 1) * per, n)
        ps = np.zeros((P * DQ, 4), np.float32)
        ts = np.zeros((P * DQ, 4), np.float32)
        ps[: hi - lo] = predf[lo:hi]
        ts[: hi - lo] = targf[lo:hi]
        # [P, 4*DQ] with plane-major columns: plane i at cols [i*DQ, (i+1)*DQ)
        dlp = ps.reshape(P, DQ, 4).transpose(0, 2, 1).reshape(P, 4 * DQ)
        dlt = ts.reshape(P, DQ, 4).transpose(0, 2, 1).reshape(P, 4 * DQ)
        m = dict(
            cnt=np.ascontiguousarray(cores[c]["cnt"]),
            nod=np.ascontiguousarray(
                cores[c]["nod"].transpose(1, 0, 2).reshape(P, 7 * (NN // P))
            ),
            dlp=np.ascontiguousarray(dlp),
            dlt=np.ascontiguousarray(dlt),
        )
        for i in range(7):
            m[f"col{i}"] = np.ascontiguousarray(cores[c]["col"][i])
        in_maps.append(m)

    key = (tuple((K, m) for K, _, m in plan), NN, S, DQ)
    nc = _get_nc(key, plan, NN, S, DQ)

    if _trace_dir is not None:
        _install_ntff_hook()
        res = run_bass_kernel_spmd(
            nc, in_maps, core_ids=list(range(N_CORES)), trace=True,
            tmpdir=_trace_dir,
        )
    else:
        res = run_bass_kernel_spmd(nc, in_maps, core_ids=list(range(N_CORES)))
    LAST_RESULT = res

    tot = np.zeros(8, np.float64)
    for c in range(N_CORES):
        tot += res.results[c]["out"].astype(np.float64).sum(axis=0)
    s_vel, s_pres, s_div2, am0, am1, am2 = tot[0], tot[1], tot[2], tot[3], tot[4], tot[5]
    loss = (
        s_vel / (3 * n)
        + s_pres / n
        + LAMBDA_CONT * s_div2 / n
        + LAMBDA_MOM * (am0 + am1 + am2) / (3 * n)
    )
    return np.float32(loss)
